# revision 11
# baseline (speedup 1.0000x reference)
"""HardClusterAssigner Trainium2 kernel.

Reference computation:
    x_emb = mean_b(einsum('bsv,hs->bvh', x, W) + b)   # [V, H]
    assignments = one_hot(argmin(-l2norm(x_emb) @ l2norm(centroids).T))

Key transformations:
  1. mean over B commutes with the (linear) contraction over S:
         mean_b(x @ W.T) = (mean_b x) @ W.T
     so the 34-GFLOP batched matmul collapses to a memory-bound reduction
     of x over B (the only large data movement: 16.8MB/core).
  2. l2norm of the embedding is a positive per-row scale -> it cannot change
     the row-wise argmin, so it is skipped; the overall positive 1/B mean
     factor is likewise argmin-invariant.
  3. sim[v,c] = sum_s xm[s,v] * Mt[s,c] + bn[c] where
         Mt = W.T @ l2norm(centroids).T   [S, C]   (256KB)
         bn = B * (l2norm(centroids) @ b) [C]
     Mt/bn are tiny (67 MFLOP) and precomputed on the host, so the device
     streams ONLY x plus 256KB of Mt: no W load (2MB/core saved), no
     centroid normalize chain, no ACT table loads.

Device pipeline per core (V sharded across 8 cores, no collectives):
  - x arrives as xs[s, v, b] (b innermost); 7 s-chunks stream as single
    2MB DMAs. All x rides ONE HWDGE ring (sync): splitting a chunk
    across both rings halves per-DMA drain rate, doubling completion
    latency, which stalls issue through the 8 shared DMA-completion
    semaphore lanes. Pools are sized so no DMA is gated on recycling.
  - the tail is bounded by sem-latency + DVE time of the data arriving
    in the last few us, so granularity is shaped across the stream:
    most of the LAST s-chunk (t=7) rides the otherwise-idle scalar ring
    early (filling the DVE's ~8us startup idle at zero cost to the sync
    FIFO), chunk 6 streams as 8x0.25MB eighths whose reduces pipeline
    with their completion sems, and one final 0.25MB piece arrives
    last -> post-stream reduce work is ~1us instead of ~5us.
  - DVE tensor_reduce sums over b per tile (1 elem/cycle: 34us total,
    under the ~40us stream at 420 GB/s).
  - per s-chunk one fp32 PE matmul accumulates xm_t.T @ Mt_t into
    sim[v,c] PSUM; bias enters via a rank-1 ones x bn matmul.
  - tail: row max + is_equal -> one-hot, DMA out.
"""

import sys

for _p in ("/opt/trn_rl_repo",):
    if _p not in sys.path:
        sys.path.append(_p)

from contextlib import ExitStack

import numpy as np

import concourse.bacc as bacc
import concourse.bass as bass
import concourse.mybir as mybir
from concourse import tile
from concourse.bass_utils import run_bass_kernel_spmd

B, S, V, H, C = 64, 1024, 512, 512, 64
NCORES = 8
VL = V // NCORES  # 64 V-columns per core
P = 128
ST = S // P  # 8 s-chunks
F32 = mybir.dt.float32

_NC_CACHE = None


def build_bass() -> bass.Bass:
    nc = bacc.Bacc("TRN2", target_bir_lowering=False)

    xs = nc.declare_dram_parameter("xs", [S, VL, B], F32, isOutput=False)
    mt = nc.declare_dram_parameter("mt", [P, ST * C], F32, isOutput=False)
    bn = nc.declare_dram_parameter("bn", [1, C], F32, isOutput=False)
    out = nc.declare_dram_parameter("out", [VL, C], F32, isOutput=True)

    with tile.TileContext(nc) as tc, ExitStack() as ctx:
        consts = ctx.enter_context(tc.tile_pool(name="consts", bufs=1))
        xpool = ctx.enter_context(tc.tile_pool(name="x", bufs=7))
        qpool = ctx.enter_context(tc.tile_pool(name="xq", bufs=1))
        xmpool = ctx.enter_context(tc.tile_pool(name="xm", bufs=3))
        spool = ctx.enter_context(tc.tile_pool(name="small", bufs=1))
        psum = ctx.enter_context(tc.tile_pool(name="psum", bufs=1, space="PSUM"))

        # Mt tiled [p, (t, c)] so the DMA is fully contiguous; bn is 256B.
        mtt = consts.tile([P, ST * C], F32)
        nc.scalar.dma_start(out=mtt[:], in_=mt[:])
        bnt = consts.tile([1, C], F32)
        nc.scalar.dma_start(out=bnt[:], in_=bn[:])
        ones_row = consts.tile([1, VL], F32)
        nc.vector.memset(ones_row[:], 1.0)

        # sim[v, c] accumulates in one PSUM bank across 1 + ST matmuls.
        sim_ps = psum.tile([VL, C], F32, tag="sim")
        nc.tensor.matmul(sim_ps[:], ones_row[:], bnt[:], start=True, stop=False)

        xs_r = xs.rearrange("(t p) v b -> t p (v b)", p=P)
        xms = [
            xmpool.tile([P, VL], F32, tag=f"xm{t}", name=f"xm{t}")
            for t in range(ST)
        ]
        TL = ST - 1  # the split tail chunk

        def stream_piece(eng, t, v0, v1, tag):
            w = v1 - v0
            pool = xpool if w == VL else qpool
            xt = pool.tile([P, w * B], F32, tag=tag, name=f"xt_{tag}")
            eng.dma_start(out=xt[:], in_=xs_r[t][:, v0 * B : v1 * B])
            nc.vector.tensor_reduce(
                xms[t][:, v0:v1],
                xt[:].rearrange("p (v b) -> p v b", b=B),
                axis=mybir.AxisListType.X,
                op=mybir.AluOpType.add,
            )

        def chunk_mm(t, stop):
            nc.tensor.matmul(
                sim_ps[:],
                xms[t][:],
                mtt[:, t * C : (t + 1) * C],
                start=False,
                stop=stop,
            )

        # Most of the tail chunk rides the idle scalar ring: lands early,
        # fills DVE startup idle, costs the sync FIFO nothing.
        stream_piece(nc.scalar, TL, 0, 24, "qa")
        stream_piece(nc.scalar, TL, 24, 48, "qb")
        stream_piece(nc.scalar, TL, 48, 56, "qc")
        # Big chunks on the sync ring.
        for t in range(ST - 2):
            stream_piece(nc.sync, t, 0, VL, "xt")
            chunk_mm(t, stop=False)
        # Chunk 6 as eighths: completion sems arrive ~0.6us apart at the
        # stream end and the small reduces pipeline with them.
        E = VL // 8
        for h in range(8):
            stream_piece(nc.sync, ST - 2, h * E, (h + 1) * E, f"e{h}")
        chunk_mm(ST - 2, stop=False)
        # Final small piece arrives last.
        stream_piece(nc.sync, TL, 56, 64, "qd")
        chunk_mm(TL, stop=True)

        # one-hot of row argmax
        mx = spool.tile([VL, 1], F32)
        nc.vector.tensor_reduce(
            mx[:], sim_ps[:], axis=mybir.AxisListType.X, op=mybir.AluOpType.max
        )
        oh = spool.tile([VL, C], F32)
        nc.vector.tensor_scalar(
            oh[:], sim_ps[:], mx[:], None, op0=mybir.AluOpType.is_equal
        )
        nc.sync.dma_start(out=out[:], in_=oh[:])

    nc.compile()
    return nc


def _get_nc() -> bass.Bass:
    global _NC_CACHE
    if _NC_CACHE is None:
        _NC_CACHE = build_bass()
    return _NC_CACHE


def make_in_maps(x, W, b, centroids):
    x = np.asarray(x, dtype=np.float32)
    W = np.asarray(W, dtype=np.float64)
    b = np.asarray(b, dtype=np.float64)
    centroids = np.asarray(centroids, dtype=np.float64)

    # Host precompute of the tiny [S, C] similarity projector (67 MFLOP):
    #   cn = l2norm(centroids); Mt = (cn @ W).T; bn = B * (cn @ b)
    cn = centroids / np.maximum(
        np.linalg.norm(centroids, axis=1, keepdims=True), 1e-12
    )
    Mt = np.ascontiguousarray((cn @ W).T)  # [S, C] float64
    # device layout [p, (t, c)] with s = t*128 + p
    mt_host = np.ascontiguousarray(
        Mt.reshape(ST, P, C).transpose(1, 0, 2)
    ).reshape(P, ST * C).astype(np.float32)
    bn_host = (np.float64(B) * (cn @ b)).reshape(1, C).astype(np.float32)

    # Two-step host transpose [B,S,V] -> [S,V,B]: one pass to [S,B,V]
    # (contiguous 2KB runs, fast), then per-s [B,VL] -> [VL,B] blocks that
    # stay cache-resident. Direct one-shot transpose would thrash DRAM.
    xsb = np.ascontiguousarray(x.transpose(1, 0, 2))  # [S, B, V]
    in_maps = []
    for i in range(NCORES):
        xs_i = np.ascontiguousarray(
            xsb[:, :, i * VL : (i + 1) * VL].transpose(0, 2, 1)
        )  # [S, VL, B]
        in_maps.append({"xs": xs_i, "mt": mt_host, "bn": bn_host})
    return in_maps


def run(inputs: dict, trace: bool = False):
    """Run on the 8 NeuronCores; returns (full_output, BassKernelResults)."""
    nc = _get_nc()
    in_maps = make_in_maps(**inputs)
    res = run_bass_kernel_spmd(nc, in_maps, list(range(NCORES)), trace=trace)
    full = np.concatenate([r["out"] for r in res.results], axis=0)
    return full, res


def kernel(x, W, b, centroids) -> np.ndarray:
    full, _ = run({"x": x, "W": W, "b": b, "centroids": centroids})
    return full


# revision 12
# speedup vs baseline: 1.0621x; 1.0621x over previous
"""HardClusterAssigner Trainium2 kernel.

Reference computation:
    x_emb = mean_b(einsum('bsv,hs->bvh', x, W) + b)   # [V, H]
    assignments = one_hot(argmin(-l2norm(x_emb) @ l2norm(centroids).T))

Key transformations:
  1. mean over B commutes with the (linear) contraction over S:
         mean_b(x @ W.T) = (mean_b x) @ W.T
     so the 34-GFLOP batched matmul collapses to a memory-bound reduction
     of x over B (the only large data movement: 16.8MB/core).
  2. l2norm of the embedding is a positive per-row scale -> it cannot change
     the row-wise argmin, so it is skipped; the overall positive 1/B mean
     factor is likewise argmin-invariant.
  3. sim[v,c] = sum_s xm[s,v] * Mt[s,c] + bn[c] where
         Mt = W.T @ l2norm(centroids).T   [S, C]   (256KB)
         bn = B * (l2norm(centroids) @ b) [C]
     Mt/bn are tiny (67 MFLOP) and precomputed on the host, so the device
     streams ONLY x plus 256KB of Mt: no W load (2MB/core saved), no
     centroid normalize chain, no ACT table loads.

Device pipeline per core (V sharded across 8 cores, no collectives):
  - x arrives as xs[s, v, b] (b innermost); 7 s-chunks stream as single
    2MB DMAs. All x rides ONE HWDGE ring (sync): splitting a chunk
    across both rings halves per-DMA drain rate, doubling completion
    latency, which stalls issue through the 8 shared DMA-completion
    semaphore lanes. Pools are sized so no DMA is gated on recycling.
  - the tail is bounded by sem-latency + DVE time of the data arriving
    in the last few us, so granularity is shaped across the stream:
    most of the LAST s-chunk (t=7) rides the otherwise-idle scalar ring
    early (filling the DVE's ~8us startup idle at zero cost to the sync
    FIFO), chunk 6 streams as 8x0.25MB eighths whose reduces pipeline
    with their completion sems, and one final 0.25MB piece arrives
    last -> post-stream reduce work is ~1us instead of ~5us.
  - DVE tensor_reduce sums over b per tile (1 elem/cycle: 34us total,
    under the ~40us stream at 420 GB/s).
  - per s-chunk one fp32 PE matmul accumulates xm_t.T @ Mt_t into
    sim[v,c] PSUM; bias enters via a rank-1 ones x bn matmul.
  - tail: row max + is_equal -> one-hot, DMA out.
"""

import sys

for _p in ("/opt/trn_rl_repo",):
    if _p not in sys.path:
        sys.path.append(_p)

from contextlib import ExitStack

import numpy as np

import concourse.bacc as bacc
import concourse.bass as bass
import concourse.mybir as mybir
from concourse import tile
from concourse.bass_utils import run_bass_kernel_spmd

B, S, V, H, C = 64, 1024, 512, 512, 64
NCORES = 8
VL = V // NCORES  # 64 V-columns per core
P = 128
ST = S // P  # 8 s-chunks
F32 = mybir.dt.float32

_NC_CACHE = None


def build_bass() -> bass.Bass:
    nc = bacc.Bacc("TRN2", target_bir_lowering=False)

    xs = nc.declare_dram_parameter("xs", [S, VL, B], F32, isOutput=False)
    mt = nc.declare_dram_parameter("mt", [P, ST * C], F32, isOutput=False)
    bn = nc.declare_dram_parameter("bn", [1, C], F32, isOutput=False)
    out = nc.declare_dram_parameter("out", [VL, C], F32, isOutput=True)

    with tile.TileContext(nc) as tc, ExitStack() as ctx:
        consts = ctx.enter_context(tc.tile_pool(name="consts", bufs=1))
        xpool = ctx.enter_context(tc.tile_pool(name="x", bufs=7))
        qpool = ctx.enter_context(tc.tile_pool(name="xq", bufs=1))
        xmpool = ctx.enter_context(tc.tile_pool(name="xm", bufs=3))
        spool = ctx.enter_context(tc.tile_pool(name="small", bufs=1))
        psum = ctx.enter_context(tc.tile_pool(name="psum", bufs=1, space="PSUM"))

        # Mt tiled [p, (t, c)] so the DMA is fully contiguous; bn is 256B.
        mtt = consts.tile([P, ST * C], F32)
        nc.scalar.dma_start(out=mtt[:], in_=mt[:])
        bnt = consts.tile([1, C], F32)
        nc.scalar.dma_start(out=bnt[:], in_=bn[:])
        ones_row = consts.tile([1, VL], F32)
        nc.vector.memset(ones_row[:], 1.0)

        # sim[v, c] accumulates in one PSUM bank across 1 + ST matmuls.
        sim_ps = psum.tile([VL, C], F32, tag="sim")
        nc.tensor.matmul(sim_ps[:], ones_row[:], bnt[:], start=True, stop=False)

        xs_r = xs.rearrange("(t p) v b -> t p (v b)", p=P)
        xms = [
            xmpool.tile([P, VL], F32, tag=f"xm{t}", name=f"xm{t}")
            for t in range(ST)
        ]
        TL = ST - 1  # the split tail chunk

        def stream_piece(eng, t, v0, v1, tag):
            w = v1 - v0
            pool = xpool if w == VL else qpool
            xt = pool.tile([P, w * B], F32, tag=tag, name=f"xt_{tag}")
            eng.dma_start(out=xt[:], in_=xs_r[t][:, v0 * B : v1 * B])
            nc.vector.tensor_reduce(
                xms[t][:, v0:v1],
                xt[:].rearrange("p (v b) -> p v b", b=B),
                axis=mybir.AxisListType.X,
                op=mybir.AluOpType.add,
            )

        def chunk_mm(t, stop):
            nc.tensor.matmul(
                sim_ps[:],
                xms[t][:],
                mtt[:, t * C : (t + 1) * C],
                start=False,
                stop=stop,
            )

        # Chunk 7 rides the otherwise-idle scalar ring: lands early while
        # the sync pipe ramps, fills the DVE's startup idle, and its MM
        # fires mid-stream instead of in the tail.
        stream_piece(nc.scalar, TL, 0, 32, "qa")
        stream_piece(nc.scalar, TL, 32, 56, "qb")
        stream_piece(nc.scalar, TL, 56, 64, "qc")
        chunk_mm(TL, stop=False)
        # Six big chunks on the sync ring keep the descriptor queue deep.
        for t in range(ST - 2):
            stream_piece(nc.sync, t, 0, VL, "xt")
            chunk_mm(t, stop=False)
        # Chunk 6 tapers 1/0.5/0.25/0.25MB so its completion sems and the
        # small reduces pipeline at the stream end; only the last quarter
        # MM (v 32:64, PSUM partitions 32:64) trails the stream.
        T6 = ST - 2
        stream_piece(nc.sync, T6, 0, 32, "t1")
        nc.tensor.matmul(
            sim_ps[0:32, :],
            xms[T6][:, 0:32],
            mtt[:, T6 * C : (T6 + 1) * C],
            start=False,
            stop=False,
        )
        stream_piece(nc.sync, T6, 32, 48, "t2")
        stream_piece(nc.sync, T6, 48, 56, "t3")
        stream_piece(nc.sync, T6, 56, 64, "t4")
        nc.tensor.matmul(
            sim_ps[32:64, :],
            xms[T6][:, 32:64],
            mtt[:, T6 * C : (T6 + 1) * C],
            start=False,
            stop=True,
        )

        # one-hot of row argmax
        mx = spool.tile([VL, 1], F32)
        nc.vector.tensor_reduce(
            mx[:], sim_ps[:], axis=mybir.AxisListType.X, op=mybir.AluOpType.max
        )
        oh = spool.tile([VL, C], F32)
        nc.vector.tensor_scalar(
            oh[:], sim_ps[:], mx[:], None, op0=mybir.AluOpType.is_equal
        )
        nc.sync.dma_start(out=out[:], in_=oh[:])

    nc.compile()
    return nc


def _get_nc() -> bass.Bass:
    global _NC_CACHE
    if _NC_CACHE is None:
        _NC_CACHE = build_bass()
    return _NC_CACHE


def make_in_maps(x, W, b, centroids):
    x = np.asarray(x, dtype=np.float32)
    W = np.asarray(W, dtype=np.float64)
    b = np.asarray(b, dtype=np.float64)
    centroids = np.asarray(centroids, dtype=np.float64)

    # Host precompute of the tiny [S, C] similarity projector (67 MFLOP):
    #   cn = l2norm(centroids); Mt = (cn @ W).T; bn = B * (cn @ b)
    cn = centroids / np.maximum(
        np.linalg.norm(centroids, axis=1, keepdims=True), 1e-12
    )
    Mt = np.ascontiguousarray((cn @ W).T)  # [S, C] float64
    # device layout [p, (t, c)] with s = t*128 + p
    mt_host = np.ascontiguousarray(
        Mt.reshape(ST, P, C).transpose(1, 0, 2)
    ).reshape(P, ST * C).astype(np.float32)
    bn_host = (np.float64(B) * (cn @ b)).reshape(1, C).astype(np.float32)

    # Two-step host transpose [B,S,V] -> [S,V,B]: one pass to [S,B,V]
    # (contiguous 2KB runs, fast), then per-s [B,VL] -> [VL,B] blocks that
    # stay cache-resident. Direct one-shot transpose would thrash DRAM.
    xsb = np.ascontiguousarray(x.transpose(1, 0, 2))  # [S, B, V]
    in_maps = []
    for i in range(NCORES):
        xs_i = np.ascontiguousarray(
            xsb[:, :, i * VL : (i + 1) * VL].transpose(0, 2, 1)
        )  # [S, VL, B]
        in_maps.append({"xs": xs_i, "mt": mt_host, "bn": bn_host})
    return in_maps


def run(inputs: dict, trace: bool = False):
    """Run on the 8 NeuronCores; returns (full_output, BassKernelResults)."""
    nc = _get_nc()
    in_maps = make_in_maps(**inputs)
    res = run_bass_kernel_spmd(nc, in_maps, list(range(NCORES)), trace=trace)
    full = np.concatenate([r["out"] for r in res.results], axis=0)
    return full, res


def kernel(x, W, b, centroids) -> np.ndarray:
    full, _ = run({"x": x, "W": W, "b": b, "centroids": centroids})
    return full


# revision 13
# speedup vs baseline: 1.3613x; 1.2817x over previous
"""HardClusterAssigner Trainium2 kernel.

Reference computation:
    x_emb = mean_b(einsum('bsv,hs->bvh', x, W) + b)   # [V, H]
    assignments = one_hot(argmin(-l2norm(x_emb) @ l2norm(centroids).T))

Key transformations:
  1. mean over B commutes with the (linear) contraction over S:
         mean_b(x @ W.T) = (mean_b x) @ W.T
     so the 34-GFLOP batched matmul collapses to a memory-bound reduction
     of x over B (the only large data movement: 16.8MB/core).
  2. l2norm of the embedding is a positive per-row scale -> it cannot change
     the row-wise argmin, so it is skipped; the overall positive 1/B mean
     factor is likewise argmin-invariant.
  3. sim[v,c] = sum_s xm[s,v] * Mt[s,c] + bn[c] where
         Mt = W.T @ l2norm(centroids).T   [S, C]   (256KB)
         bn = B * (l2norm(centroids) @ b) [C]
     Mt/bn are tiny (67 MFLOP) and precomputed on the host, so the device
     streams ONLY x plus 256KB of Mt: no W load (2MB/core saved), no
     centroid normalize chain, no ACT table loads.

Device pipeline per core (V sharded across 8 cores, no collectives):
  - x arrives as xs[s, v, b] (b innermost); 7 s-chunks stream as single
    2MB DMAs. All x rides ONE HWDGE ring (sync): splitting a chunk
    across both rings halves per-DMA drain rate, doubling completion
    latency, which stalls issue through the 8 shared DMA-completion
    semaphore lanes. Pools are sized so no DMA is gated on recycling.
  - the tail is bounded by sem-latency + DVE time of the data arriving
    in the last few us, so granularity is shaped across the stream:
    most of the LAST s-chunk (t=7) rides the otherwise-idle scalar ring
    early (filling the DVE's ~8us startup idle at zero cost to the sync
    FIFO), chunk 6 streams as 8x0.25MB eighths whose reduces pipeline
    with their completion sems, and one final 0.25MB piece arrives
    last -> post-stream reduce work is ~1us instead of ~5us.
  - DVE tensor_reduce sums over b per tile (1 elem/cycle: 34us total,
    under the ~40us stream at 420 GB/s).
  - per s-chunk one fp32 PE matmul accumulates xm_t.T @ Mt_t into
    sim[v,c] PSUM; bias enters via a rank-1 ones x bn matmul.
  - tail: row max + is_equal -> one-hot, DMA out.
"""

import sys

for _p in ("/opt/trn_rl_repo",):
    if _p not in sys.path:
        sys.path.append(_p)

from contextlib import ExitStack

import numpy as np

import concourse.bacc as bacc
import concourse.bass as bass
import concourse.mybir as mybir
from concourse import tile
from concourse.bass_utils import run_bass_kernel_spmd

B, S, V, H, C = 64, 1024, 512, 512, 64
NCORES = 8
VL = V // NCORES  # 64 V-columns per core
P = 128
ST = S // P  # 8 s-chunks
F32 = mybir.dt.float32

_NC_CACHE = None


def build_bass() -> bass.Bass:
    nc = bacc.Bacc("TRN2", target_bir_lowering=False)

    xs = nc.declare_dram_parameter("xs", [S, VL, B], F32, isOutput=False)
    mt = nc.declare_dram_parameter("mt", [P, ST * C], F32, isOutput=False)
    bn = nc.declare_dram_parameter("bn", [1, C], F32, isOutput=False)
    out = nc.declare_dram_parameter("out", [VL, C], F32, isOutput=True)

    with tile.TileContext(nc) as tc, ExitStack() as ctx:
        consts = ctx.enter_context(tc.tile_pool(name="consts", bufs=1))
        xpool = ctx.enter_context(tc.tile_pool(name="x", bufs=7))
        qpool = ctx.enter_context(tc.tile_pool(name="xq", bufs=1))
        xmpool = ctx.enter_context(tc.tile_pool(name="xm", bufs=3))
        spool = ctx.enter_context(tc.tile_pool(name="small", bufs=1))
        psum = ctx.enter_context(tc.tile_pool(name="psum", bufs=1, space="PSUM"))

        # Mt tiled [p, (t, c)] so the DMA is fully contiguous; bn is 256B.
        mtt = consts.tile([P, ST * C], F32)
        nc.scalar.dma_start(out=mtt[:], in_=mt[:])
        bnt = consts.tile([1, C], F32)
        nc.scalar.dma_start(out=bnt[:], in_=bn[:])
        ones_row = consts.tile([1, VL], F32)
        nc.vector.memset(ones_row[:], 1.0)

        # sim[v, c] accumulates in one PSUM bank across 1 + ST matmuls.
        sim_ps = psum.tile([VL, C], F32, tag="sim")
        nc.tensor.matmul(sim_ps[:], ones_row[:], bnt[:], start=True, stop=False)

        xs_r = xs.rearrange("(t p) v b -> t p (v b)", p=P)
        xms = [
            xmpool.tile([P, VL], F32, tag=f"xm{t}", name=f"xm{t}")
            for t in range(ST)
        ]
        TL = ST - 1  # the split tail chunk

        def stream_piece(eng, t, v0, v1, tag):
            w = v1 - v0
            pool = xpool if w == VL else qpool
            xt = pool.tile([P, w * B], F32, tag=tag, name=f"xt_{tag}")
            eng.dma_start(out=xt[:], in_=xs_r[t][:, v0 * B : v1 * B])
            nc.vector.tensor_reduce(
                xms[t][:, v0:v1],
                xt[:].rearrange("p (v b) -> p v b", b=B),
                axis=mybir.AxisListType.X,
                op=mybir.AluOpType.add,
            )

        def chunk_mm(t, stop):
            nc.tensor.matmul(
                sim_ps[:],
                xms[t][:],
                mtt[:, t * C : (t + 1) * C],
                start=False,
                stop=stop,
            )

        def half_mm(t, half, stop):
            v0 = half * 32
            nc.tensor.matmul(
                sim_ps[v0 : v0 + 32, :],
                xms[t][:, v0 : v0 + 32],
                mtt[:, t * C : (t + 1) * C],
                start=False,
                stop=stop,
            )

        # Six big chunks on the one ring keep the descriptor queue deep.
        # Total DMA count stays <=16 so no semaphore lane ever gets a 3rd
        # user: a >=32 lane wait targets a mid-stream DMA whose completion
        # sem lags ~5-10us under load and starves the stream (measured).
        for t in range(ST - 2):
            stream_piece(nc.sync, t, 0, VL, "xt")
            chunk_mm(t, stop=False)
        # Last two chunks taper so the post-stream serial DVE work (~9us
        # for 4MB) overlaps their arrival; trailing MMs split by v-half
        # (PSUM partitions 0:32 / 32:64) so only a half-MM trails.
        T6 = ST - 2
        stream_piece(nc.sync, T6, 0, 32, "t1")
        half_mm(T6, 0, stop=False)
        stream_piece(nc.sync, T6, 32, 48, "t2")
        stream_piece(nc.sync, T6, 48, 64, "t3")
        half_mm(T6, 1, stop=False)
        stream_piece(nc.sync, TL, 0, 24, "u1")
        stream_piece(nc.sync, TL, 24, 48, "u2")
        half_mm(TL, 0, stop=False)
        stream_piece(nc.sync, TL, 48, 56, "u3")
        stream_piece(nc.sync, TL, 56, 64, "u4")
        half_mm(TL, 1, stop=True)

        # one-hot of row argmax
        mx = spool.tile([VL, 1], F32)
        nc.vector.tensor_reduce(
            mx[:], sim_ps[:], axis=mybir.AxisListType.X, op=mybir.AluOpType.max
        )
        oh = spool.tile([VL, C], F32)
        nc.vector.tensor_scalar(
            oh[:], sim_ps[:], mx[:], None, op0=mybir.AluOpType.is_equal
        )
        nc.sync.dma_start(out=out[:], in_=oh[:])

    nc.compile()
    return nc


def _get_nc() -> bass.Bass:
    global _NC_CACHE
    if _NC_CACHE is None:
        _NC_CACHE = build_bass()
    return _NC_CACHE


def make_in_maps(x, W, b, centroids):
    x = np.asarray(x, dtype=np.float32)
    W = np.asarray(W, dtype=np.float64)
    b = np.asarray(b, dtype=np.float64)
    centroids = np.asarray(centroids, dtype=np.float64)

    # Host precompute of the tiny [S, C] similarity projector (67 MFLOP):
    #   cn = l2norm(centroids); Mt = (cn @ W).T; bn = B * (cn @ b)
    cn = centroids / np.maximum(
        np.linalg.norm(centroids, axis=1, keepdims=True), 1e-12
    )
    Mt = np.ascontiguousarray((cn @ W).T)  # [S, C] float64
    # device layout [p, (t, c)] with s = t*128 + p
    mt_host = np.ascontiguousarray(
        Mt.reshape(ST, P, C).transpose(1, 0, 2)
    ).reshape(P, ST * C).astype(np.float32)
    bn_host = (np.float64(B) * (cn @ b)).reshape(1, C).astype(np.float32)

    # Two-step host transpose [B,S,V] -> [S,V,B]: one pass to [S,B,V]
    # (contiguous 2KB runs, fast), then per-s [B,VL] -> [VL,B] blocks that
    # stay cache-resident. Direct one-shot transpose would thrash DRAM.
    xsb = np.ascontiguousarray(x.transpose(1, 0, 2))  # [S, B, V]
    in_maps = []
    for i in range(NCORES):
        xs_i = np.ascontiguousarray(
            xsb[:, :, i * VL : (i + 1) * VL].transpose(0, 2, 1)
        )  # [S, VL, B]
        in_maps.append({"xs": xs_i, "mt": mt_host, "bn": bn_host})
    return in_maps


def run(inputs: dict, trace: bool = False):
    """Run on the 8 NeuronCores; returns (full_output, BassKernelResults)."""
    nc = _get_nc()
    in_maps = make_in_maps(**inputs)
    res = run_bass_kernel_spmd(nc, in_maps, list(range(NCORES)), trace=trace)
    full = np.concatenate([r["out"] for r in res.results], axis=0)
    return full, res


def kernel(x, W, b, centroids) -> np.ndarray:
    full, _ = run({"x": x, "W": W, "b": b, "centroids": centroids})
    return full


# revision 14
# speedup vs baseline: 1.3994x; 1.0280x over previous
"""HardClusterAssigner Trainium2 kernel.

Reference computation:
    x_emb = mean_b(einsum('bsv,hs->bvh', x, W) + b)   # [V, H]
    assignments = one_hot(argmin(-l2norm(x_emb) @ l2norm(centroids).T))

Key transformations:
  1. mean over B commutes with the (linear) contraction over S:
         mean_b(x @ W.T) = (mean_b x) @ W.T
     so the 34-GFLOP batched matmul collapses to a memory-bound reduction
     of x over B (the only large data movement: 16.8MB/core).
  2. l2norm of the embedding is a positive per-row scale -> it cannot change
     the row-wise argmin, so it is skipped; the overall positive 1/B mean
     factor is likewise argmin-invariant.
  3. sim[v,c] = sum_s xm[s,v] * Mt[s,c] + bn[c] where
         Mt = W.T @ l2norm(centroids).T   [S, C]   (256KB)
         bn = B * (l2norm(centroids) @ b) [C]
     Mt/bn are tiny (67 MFLOP) and precomputed on the host, so the device
     streams ONLY x plus 256KB of Mt: no W load (2MB/core saved), no
     centroid normalize chain, no ACT table loads.

Device pipeline per core (V sharded across 8 cores, no collectives):
  - x arrives as xs[s, v, b] (b innermost); 7 s-chunks stream as single
    2MB DMAs. All x rides ONE HWDGE ring (sync): splitting a chunk
    across both rings halves per-DMA drain rate, doubling completion
    latency, which stalls issue through the 8 shared DMA-completion
    semaphore lanes. Pools are sized so no DMA is gated on recycling.
  - the tail is bounded by sem-latency + DVE time of the data arriving
    in the last few us, so granularity is shaped across the stream:
    most of the LAST s-chunk (t=7) rides the otherwise-idle scalar ring
    early (filling the DVE's ~8us startup idle at zero cost to the sync
    FIFO), chunk 6 streams as 8x0.25MB eighths whose reduces pipeline
    with their completion sems, and one final 0.25MB piece arrives
    last -> post-stream reduce work is ~1us instead of ~5us.
  - DVE tensor_reduce sums over b per tile (1 elem/cycle: 34us total,
    under the ~40us stream at 420 GB/s).
  - per s-chunk one fp32 PE matmul accumulates xm_t.T @ Mt_t into
    sim[v,c] PSUM; bias enters via a rank-1 ones x bn matmul.
  - tail: row max + is_equal -> one-hot, DMA out.
"""

import sys

for _p in ("/opt/trn_rl_repo",):
    if _p not in sys.path:
        sys.path.append(_p)

from contextlib import ExitStack

import numpy as np

import concourse.bacc as bacc
import concourse.bass as bass
import concourse.mybir as mybir
from concourse import tile
from concourse.bass_utils import run_bass_kernel_spmd

B, S, V, H, C = 64, 1024, 512, 512, 64
NCORES = 8
VL = V // NCORES  # 64 V-columns per core
P = 128
ST = S // P  # 8 s-chunks
F32 = mybir.dt.float32

_NC_CACHE = None


def build_bass() -> bass.Bass:
    nc = bacc.Bacc("TRN2", target_bir_lowering=False)

    xs = nc.declare_dram_parameter("xs", [S, VL, B], F32, isOutput=False)
    mt = nc.declare_dram_parameter("mt", [P, ST * C], F32, isOutput=False)
    bn = nc.declare_dram_parameter("bn", [1, C], F32, isOutput=False)
    out = nc.declare_dram_parameter("out", [VL, C], F32, isOutput=True)

    with tile.TileContext(nc) as tc, ExitStack() as ctx:
        consts = ctx.enter_context(tc.tile_pool(name="consts", bufs=1))
        xpool = ctx.enter_context(tc.tile_pool(name="x", bufs=7))
        qpool = ctx.enter_context(tc.tile_pool(name="xq", bufs=1))
        xmpool = ctx.enter_context(tc.tile_pool(name="xm", bufs=3))
        spool = ctx.enter_context(tc.tile_pool(name="small", bufs=1))
        psum = ctx.enter_context(tc.tile_pool(name="psum", bufs=1, space="PSUM"))

        # Mt tiled [p, (t, c)] so the DMA is fully contiguous; bn is 256B.
        mtt = consts.tile([P, ST * C], F32)
        nc.scalar.dma_start(out=mtt[:], in_=mt[:])
        bnt = consts.tile([1, C], F32)
        nc.scalar.dma_start(out=bnt[:], in_=bn[:])
        ones_row = consts.tile([1, VL], F32)
        nc.vector.memset(ones_row[:], 1.0)

        # sim[v, c] accumulates in one PSUM bank across 1 + ST matmuls.
        sim_ps = psum.tile([VL, C], F32, tag="sim")
        nc.tensor.matmul(sim_ps[:], ones_row[:], bnt[:], start=True, stop=False)

        xs_r = xs.rearrange("(t p) v b -> t p (v b)", p=P)
        xms = [
            xmpool.tile([P, VL], F32, tag=f"xm{t}", name=f"xm{t}")
            for t in range(ST)
        ]
        TL = ST - 1  # the split tail chunk

        def stream_piece(eng, t, v0, v1, tag):
            w = v1 - v0
            pool = xpool if w == VL else qpool
            xt = pool.tile([P, w * B], F32, tag=tag, name=f"xt_{tag}")
            eng.dma_start(out=xt[:], in_=xs_r[t][:, v0 * B : v1 * B])
            nc.vector.tensor_reduce(
                xms[t][:, v0:v1],
                xt[:].rearrange("p (v b) -> p v b", b=B),
                axis=mybir.AxisListType.X,
                op=mybir.AluOpType.add,
            )

        def chunk_mm(t, stop):
            nc.tensor.matmul(
                sim_ps[:],
                xms[t][:],
                mtt[:, t * C : (t + 1) * C],
                start=False,
                stop=stop,
            )

        def half_mm(t, half, stop):
            v0 = half * 32
            nc.tensor.matmul(
                sim_ps[v0 : v0 + 32, :],
                xms[t][:, v0 : v0 + 32],
                mtt[:, t * C : (t + 1) * C],
                start=False,
                stop=stop,
            )

        # Ring model (measured): the HWDGE ring holds ~4 in-flight DMA
        # requests; issue k waits the completion sem of k-4, and sems lag
        # data by ~4us under load. Early chunks stay 2MB (deep queue);
        # C4/C5 split into 1MB halves so completion sems arrive at fine
        # granularity over the last 8MB and the DVE tracks arrivals
        # instead of serializing a 13us tail after C5's laggy sem.
        for t in range(4):
            stream_piece(nc.sync, t, 0, VL, "xt")
            chunk_mm(t, stop=False)
        for t in (4, 5):
            stream_piece(nc.sync, t, 0, 32, f"xh{t}a")
            stream_piece(nc.sync, t, 32, 64, f"xh{t}b")
            chunk_mm(t, stop=False)
        # Last two chunks taper so the post-stream serial DVE work (~9us
        # for 4MB) overlaps their arrival; trailing MMs split by v-half
        # (PSUM partitions 0:32 / 32:64) so only a half-MM trails.
        T6 = ST - 2
        stream_piece(nc.sync, T6, 0, 32, "t1")
        half_mm(T6, 0, stop=False)
        stream_piece(nc.sync, T6, 32, 48, "t2")
        stream_piece(nc.sync, T6, 48, 64, "t3")
        half_mm(T6, 1, stop=False)
        stream_piece(nc.sync, TL, 0, 24, "u1")
        stream_piece(nc.sync, TL, 24, 48, "u2")
        half_mm(TL, 0, stop=False)
        stream_piece(nc.sync, TL, 48, 56, "u3")
        stream_piece(nc.sync, TL, 56, 64, "u4")
        half_mm(TL, 1, stop=True)

        # one-hot of row argmax
        mx = spool.tile([VL, 1], F32)
        nc.vector.tensor_reduce(
            mx[:], sim_ps[:], axis=mybir.AxisListType.X, op=mybir.AluOpType.max
        )
        oh = spool.tile([VL, C], F32)
        nc.vector.tensor_scalar(
            oh[:], sim_ps[:], mx[:], None, op0=mybir.AluOpType.is_equal
        )
        nc.sync.dma_start(out=out[:], in_=oh[:])

    nc.compile()
    return nc


def _get_nc() -> bass.Bass:
    global _NC_CACHE
    if _NC_CACHE is None:
        _NC_CACHE = build_bass()
    return _NC_CACHE


def make_in_maps(x, W, b, centroids):
    x = np.asarray(x, dtype=np.float32)
    W = np.asarray(W, dtype=np.float64)
    b = np.asarray(b, dtype=np.float64)
    centroids = np.asarray(centroids, dtype=np.float64)

    # Host precompute of the tiny [S, C] similarity projector (67 MFLOP):
    #   cn = l2norm(centroids); Mt = (cn @ W).T; bn = B * (cn @ b)
    cn = centroids / np.maximum(
        np.linalg.norm(centroids, axis=1, keepdims=True), 1e-12
    )
    Mt = np.ascontiguousarray((cn @ W).T)  # [S, C] float64
    # device layout [p, (t, c)] with s = t*128 + p
    mt_host = np.ascontiguousarray(
        Mt.reshape(ST, P, C).transpose(1, 0, 2)
    ).reshape(P, ST * C).astype(np.float32)
    bn_host = (np.float64(B) * (cn @ b)).reshape(1, C).astype(np.float32)

    # Two-step host transpose [B,S,V] -> [S,V,B]: one pass to [S,B,V]
    # (contiguous 2KB runs, fast), then per-s [B,VL] -> [VL,B] blocks that
    # stay cache-resident. Direct one-shot transpose would thrash DRAM.
    xsb = np.ascontiguousarray(x.transpose(1, 0, 2))  # [S, B, V]
    in_maps = []
    for i in range(NCORES):
        xs_i = np.ascontiguousarray(
            xsb[:, :, i * VL : (i + 1) * VL].transpose(0, 2, 1)
        )  # [S, VL, B]
        in_maps.append({"xs": xs_i, "mt": mt_host, "bn": bn_host})
    return in_maps


def run(inputs: dict, trace: bool = False):
    """Run on the 8 NeuronCores; returns (full_output, BassKernelResults)."""
    nc = _get_nc()
    in_maps = make_in_maps(**inputs)
    res = run_bass_kernel_spmd(nc, in_maps, list(range(NCORES)), trace=trace)
    full = np.concatenate([r["out"] for r in res.results], axis=0)
    return full, res


def kernel(x, W, b, centroids) -> np.ndarray:
    full, _ = run({"x": x, "W": W, "b": b, "centroids": centroids})
    return full
